# revision 27
# baseline (speedup 1.0000x reference)
"""Trainium2 Bass kernel for log-softmax multi-head attention (8 NeuronCores).

Reference computation (per batch):
    qkv = x @ w_qkv ; q,k,v per head
    dots = scale * q @ k^T ; attn = log_softmax(dots)
    out = attn @ v  -> merge heads -> out @ w_out + b_out + x

Key algebraic identity: log_softmax is linear in the scores minus a row
constant:  attn = scale*dots - lse  with  lse_i = logsumexp_j(scale*dots_ij).
Therefore
    out_head = scale * q @ (k^T v) - lse (x) colsum(v)
which removes the O(n^2 d) attention-apply; only the lse pass is O(n^2).

Sharding: 8 cores = 2 batches x 4 query-quarters. Every core computes k/v for
its full batch (duplicated across the 4 cores of a batch) and q / lse / output
for its own 1024 query rows -> outputs are disjoint, no collectives.

Engine/schedule strategy:
- x^T arrives via big DMA xbar transposes of the bf16 copy of x (bf16 cast is
  host-side input prep; f32 x is still used for the residual).
- All n^2-sized matmuls are bf16 (1 PE cycle/row); the lse-correction rank-1
  term stays fp32; PSUM accumulation is always fp32.
- The exp+row-sum pass (ScalarE, activation with accum_out over PSUM score
  tiles) is the ~292us critical path. PSUM is time-shared so the second half
  of the k/v/kT production overlaps it: during that window the score pipeline
  runs single-buffered on one 4-bank pool (ScalarE pays a small stall per
  group) while k/v production keeps its own 4 banks; afterwards a second
  4-bank pool restores fully double-buffered back-to-back exps.
"""

import numpy as np

B, N, D = 2, 4096, 512
H, DH = 8, 64
SCALE = DH**-0.5
NQ = N // 4  # own query rows per core
NT = N // 128  # 32 key tiles
QT = NQ // 128  # 8 own row tiles

_GRAPH_CACHE = {}


def _build_graph():
    import concourse.bass as bass
    import concourse.tile as tile
    from concourse import bacc, mybir
    from concourse.masks import make_identity

    f32 = mybir.dt.float32
    bf16 = mybir.dt.bfloat16
    AF = mybir.ActivationFunctionType

    nc = bacc.Bacc("TRN2", target_bir_lowering=False, debug=False)

    xbf_d = nc.dram_tensor("x_bf", [N, D], bf16, kind="ExternalInput").ap()
    xqbf_d = nc.dram_tensor("xq_bf", [NQ, D], bf16, kind="ExternalInput").ap()
    xq_d = nc.dram_tensor("xq", [NQ, D], f32, kind="ExternalInput").ap()
    wqkv_d = nc.dram_tensor("w_qkv", [D, 3 * D], f32, kind="ExternalInput").ap()
    wout_d = nc.dram_tensor("w_out", [D, D], f32, kind="ExternalInput").ap()
    bout_d = nc.dram_tensor("b_out", [D], f32, kind="ExternalInput").ap()
    out_d = nc.dram_tensor("out", [NQ, D], f32, kind="ExternalOutput").ap()

    with tile.TileContext(nc) as tc:
        with (
            tc.tile_pool(name="const", bufs=1) as const,
            tc.tile_pool(name="bigsb", bufs=1) as bigsb,
            tc.tile_pool(name="stage", bufs=3) as stage,
            tc.tile_pool(name="dout", bufs=2) as dout,
            tc.tile_pool(name="lsr", bufs=2) as lsr,
        ):
            identity = const.tile([128, 128], f32, tag="identity")
            make_identity(nc, identity[:])
            identity_bf = const.tile([128, 128], bf16, tag="identity_bf")
            make_identity(nc, identity_bf[:])
            ones_neg = const.tile([128, 1], bf16, tag="ones_neg")
            nc.vector.memset(ones_neg[:], -1.0)
            b_bc = const.tile([128, D], f32, tag="b_bc")
            nc.sync.dma_start(
                out=b_bc[:],
                in_=bass.AP(
                    tensor=bout_d.tensor,
                    offset=bout_d.offset,
                    ap=[[0, 128]] + [list(p) for p in bout_d.ap],
                ),
            )
            # weights: fast HWDGE f32 load + DVE cast to bf16
            wq = []
            for j in range(4):
                w_f = stage.tile([128, 3 * D], f32, name=f"wqf{j}", tag="wqf")
                nc.sync.dma_start(out=w_f[:], in_=wqkv_d[j * 128 : (j + 1) * 128, :])
                w_t = const.tile([128, 3 * D], bf16, name=f"wq{j}", tag=f"wq{j}")
                nc.vector.tensor_copy(w_t[:], w_f[:])
                wq.append(w_t)
            wo = []
            for j in range(4):
                w_f = stage.tile([128, D], f32, name=f"wof{j}", tag="wof")
                nc.sync.dma_start(out=w_f[:], in_=wout_d[j * 128 : (j + 1) * 128, :])
                w_t = const.tile([128, D], bf16, name=f"wo{j}", tag=f"wo{j}")
                nc.vector.tensor_copy(w_t[:], w_f[:])
                wo.append(w_t)

            # kv_acc: [0:128, 0:512]: four [128,128] head-pair blocks of k^T v
            # (pair c's diagonal 64x64 sub-blocks are heads 2c / 2c+1).
            # [0:1, 512:1024]: -colsum(v) over all 512 v columns.
            kv_acc = const.tile([128, 1024], f32, tag="kv_acc")
            nc.vector.memset(kv_acc[:], 0.0)
            # kv_p: per-head K=128 stationary operand for OT matmuls, scaled by
            # SCALE (head h's block at rows (h%2)*64 of col block h*64, zeros
            # elsewhere so contracting against the full qT partition range only
            # picks up head h's q rows).
            kv_p = const.tile([128, 512], bf16, tag="kv_p")
            nc.vector.memset(kv_p[:], 0.0)

            kT = [bigsb.tile([128, N], bf16, name=f"kT{c}", tag=f"kT{c}") for c in range(4)]
            qT = [bigsb.tile([128, NQ], bf16, name=f"qT{c}", tag=f"qT{c}") for c in range(4)]
            xT = [bigsb.tile([128, N], bf16, name=f"xT{j}", tag=f"xT{j}") for j in range(4)]
            xTq = [bigsb.tile([128, NQ], bf16, name=f"xTq{j}", tag=f"xTq{j}") for j in range(4)]
            OT = [bigsb.tile([128, NQ], bf16, name=f"OT{c}", tag=f"OT{c}") for c in range(4)]

            lse_acc = const.tile([128, 128], f32, tag="lse_acc")
            lse_sum = const.tile([128, 64], f32, tag="lse_sum")
            lse_ln = const.tile([128, 64], bf16, tag="lse_ln")
            nvs_bf = const.tile([1, 512], bf16, tag="nvs_bf")

            # score-tile pools: cpsA lives the whole kernel (also used by the
            # overlap window and phase E); cpsB is added once the k/v pool
            # releases its 4 banks. Stack order: cpsA below abps (LIFO).
            cpsA = tc.alloc_tile_pool(name="c_psA", bufs=1, space="PSUM")
            abps = tc.alloc_tile_pool(name="ab_ps", bufs=1, space="PSUM")

            # ---- phase A: x^T via big DMA xbar transposes (bf16) -------------
            # first key-row chunk first so phase B starts immediately
            for j in range(4):
                nc.sync.dma_start(
                    out=xT[j][:, 0:1024],
                    in_=xbf_d[0:1024, j * 128 : (j + 1) * 128],
                    transpose=True,
                )
            for j in range(4):
                nc.sync.dma_start(
                    out=xTq[j][:],
                    in_=xqbf_d[:, j * 128 : (j + 1) * 128],
                    transpose=True,
                )
            for r in range(1, 4):
                for j in range(4):
                    nc.sync.dma_start(
                        out=xT[j][:, r * 1024 : (r + 1) * 1024],
                        in_=xbf_d[r * 1024 : (r + 1) * 1024, j * 128 : (j + 1) * 128],
                        transpose=True,
                    )

            def ab_tile(t, copies_on_dve):
                """k/v/kv/kT production for key tile t (4 PSUM banks via
                tag 'big' bufs=2). seg0: k then head-pair kv blocks then k^T
                (bf16 view); seg1: v then -colsum(v) in row 0."""
                big = abps.tile([128, 1024], f32, name="big", tag="big", bufs=2)
                for half in range(2):
                    for j in range(4):
                        nc.tensor.matmul(
                            big[:, half * 512 : (half + 1) * 512],
                            lhsT=xT[j][:, t * 128 : (t + 1) * 128],
                            rhs=wq[j][:, 512 + half * 512 : 1024 + half * 512],
                            start=(j == 0),
                            stop=(j == 3),
                        )
                ks = stage.tile([128, D], bf16, name="k_stage", tag="k_stage")
                vs = stage.tile([128, D], bf16, name="v_stage", tag="v_stage")
                if copies_on_dve:
                    nc.vector.tensor_copy(ks[:], big[:, 0:512])
                    nc.vector.tensor_copy(vs[:], big[:, 512:1024])
                else:
                    nc.scalar.copy(ks[:], big[:, 0:512])
                    nc.scalar.copy(vs[:], big[:, 512:1024])
                # seg0 reuse <- head-pair k^T v blocks
                for p in range(4):
                    nc.tensor.matmul(
                        big[:, p * 128 : (p + 1) * 128],
                        lhsT=ks[:, p * 128 : (p + 1) * 128],
                        rhs=vs[:, p * 128 : (p + 1) * 128],
                        start=True,
                        stop=True,
                    )
                # seg1 row 0 reuse <- -colsum(v)
                nc.tensor.matmul(
                    big[0:1, 512:1024], lhsT=ones_neg[:], rhs=vs[:],
                    start=True, stop=True,
                )
                nc.vector.tensor_add(
                    kv_acc[:, 0:512], kv_acc[:, 0:512], big[:, 0:512]
                )
                nc.vector.tensor_add(
                    kv_acc[0:1, 512:1024], kv_acc[0:1, 512:1024], big[0:1, 512:1024]
                )
                # seg0 reuse (bf16 view, after the kv add) <- k^T
                seg0bf = big[:, 0:512].bitcast(bf16)
                for j in range(4):
                    nc.tensor.transpose(
                        seg0bf[:, j * 128 : (j + 1) * 128],
                        ks[:, j * 128 : (j + 1) * 128],
                        identity_bf[:],
                    )
                for j in range(4):
                    nc.vector.tensor_copy(
                        kT[j][:, t * 128 : (t + 1) * 128],
                        seg0bf[:, j * 128 : (j + 1) * 128],
                    )

            def c_group(h, t, half, pool, tag):
                """Scores + in-place exp + row-sum for one (head, row-tile,
                key-half) on the given 4-bank score pool."""
                r0 = (h % 2) * 64
                c = h // 2
                dots = pool.tile([128, 2048], f32, name="dots", tag=tag, bufs=1)
                for cc in range(4):
                    nc.tensor.matmul(
                        dots[:, cc * 512 : (cc + 1) * 512],
                        lhsT=qT[c][r0 : r0 + 64, t * 128 : (t + 1) * 128],
                        rhs=kT[c][
                            r0 : r0 + 64,
                            (half * 4 + cc) * 512 : (half * 4 + cc + 1) * 512,
                        ],
                        start=True,
                        stop=True,
                    )
                col = (h * 8 + t) * 2 + half
                nc.scalar.activation(
                    out=dots[:],
                    in_=dots[:],
                    func=AF.Exp,
                    scale=SCALE,
                    accum_out=lse_acc[:, col : col + 1],
                )

            # ---- phase B lead-in: key tiles 0-15, then qT --------------------
            for t in range(16):
                ab_tile(t, copies_on_dve=False)
            for m in range(4):
                for nn in range(2):
                    big = abps.tile([128, 1024], f32, name="big", tag="big", bufs=2)
                    for j in range(4):
                        nc.tensor.matmul(
                            big[:, 0:512],
                            lhsT=wq[j][:, m * 128 : (m + 1) * 128],
                            rhs=xTq[j][:, nn * 512 : (nn + 1) * 512],
                            start=(j == 0),
                            stop=(j == 3),
                        )
                    nc.vector.tensor_copy(
                        qT[m][:, nn * 512 : (nn + 1) * 512], big[:, 0:512]
                    )

            # ---- overlap window: key tiles 16-31 woven between the first -----
            # ---- score groups (single-buffered on cpsA) ----------------------
            half0 = [(h, t) for h in range(H) for t in range(QT)]
            gi = 0
            for i in range(16):
                ab_tile(16 + i, copies_on_dve=True)
                for _ in range(1 if i < 8 else 2):
                    h, t = half0[gi]
                    gi += 1
                    c_group(h, t, 0, cpsA, "dotsA")

            for h in range(H):
                r0 = (h % 2) * 64
                nc.vector.tensor_scalar_mul(
                    kv_p[r0 : r0 + 64, h * 64 : (h + 1) * 64],
                    kv_acc[r0 : r0 + 64, (h // 2) * 128 + r0 : (h // 2) * 128 + r0 + 64],
                    SCALE,
                )
            nc.vector.tensor_copy(nvs_bf[:], kv_acc[0:1, 512:1024])
            abps.release()
            cpsB = tc.alloc_tile_pool(name="c_psB", bufs=1, space="PSUM")

            # ---- remaining score groups: alternate the two 4-bank pools ------
            rest = [(h, t, 0) for (h, t) in half0[gi:]] + [
                (h, t, 1) for h in range(H) for t in range(QT)
            ]
            for g, (h, t, half) in enumerate(rest):
                if g % 2 == 0:
                    c_group(h, t, half, cpsB, "dotsB")
                else:
                    c_group(h, t, half, cpsA, "dotsA")

            la = lse_acc[:].rearrange("p (c two) -> p c two", two=2)
            nc.vector.tensor_add(lse_sum[:], la[:, :, 0], la[:, :, 1])
            nc.scalar.activation(out=lse_ln[:], in_=lse_sum[:], func=AF.Ln)
            cpsB.release()

            # ---- phase D/E: per-head outputs, projection, residual -----------
            dps = tc.alloc_tile_pool(name="de_ps", bufs=1, space="PSUM")
            for h in range(H):
                r0 = (h % 2) * 64
                c = h // 2
                # lse row [1, 1024] at partition 0
                lrs = lsr.tile([1, 1024], bf16, name="lrs", tag="lrs")
                lrp = dps.tile([128, 1024], bf16, name="lrp", tag="lrp", bufs=1)
                for t in range(QT):
                    nc.tensor.transpose(
                        lrp[0:1, t * 128 : (t + 1) * 128],
                        lse_ln[:, h * 8 + t : h * 8 + t + 1],
                        identity_bf[:],
                    )
                nc.vector.tensor_copy(lrs[0:1, :], lrp[0:1, :])
                # OT_h = s (kv_h)^T q_h^T - vsum_h (x) lse_h
                for nn in range(2):
                    ot = dps.tile([128, 512], f32, name="ot", tag="ot", bufs=2)
                    nc.tensor.matmul(
                        ot[r0 : r0 + 64, :],
                        lhsT=kv_p[:, h * 64 : (h + 1) * 64],
                        rhs=qT[c][:, nn * 512 : (nn + 1) * 512],
                        start=True,
                        stop=False,
                    )
                    nc.tensor.matmul(
                        ot[r0 : r0 + 64, :],
                        lhsT=nvs_bf[0:1, h * 64 : (h + 1) * 64],
                        rhs=lrs[0:1, nn * 512 : (nn + 1) * 512],
                        start=False,
                        stop=True,
                    )
                    nc.vector.tensor_copy(
                        OT[c][r0 : r0 + 64, nn * 512 : (nn + 1) * 512],
                        ot[r0 : r0 + 64, :],
                    )

            for t in range(QT):
                ybig = cpsA.tile([128, 2048], f32, name="dots", tag="dotsA", bufs=1)
                yps = ybig[:, 0:512]
                for c in range(4):
                    nc.tensor.matmul(
                        yps,
                        lhsT=OT[c][:, t * 128 : (t + 1) * 128],
                        rhs=wo[c][:],
                        start=(c == 0),
                        stop=(c == 3),
                    )
                xr = dout.tile([128, D], f32, name="xr", tag="xr")
                nc.sync.dma_start(out=xr[:], in_=xq_d[t * 128 : (t + 1) * 128, :])
                ysb = dout.tile([128, D], f32, name="ysb", tag="ysb")
                nc.vector.tensor_add(ysb[:], yps, xr[:])
                nc.vector.tensor_add(ysb[:], ysb[:], b_bc[:])
                nc.sync.dma_start(out=out_d[t * 128 : (t + 1) * 128, :], in_=ysb[:])

            dps.release()
            cpsA.release()

    nc.compile()
    return nc


def get_graph():
    if "nc" not in _GRAPH_CACHE:
        _GRAPH_CACHE["nc"] = _build_graph()
    return _GRAPH_CACHE["nc"]


def make_in_maps(x, w_qkv, w_out, b_out):
    import ml_dtypes

    x = np.ascontiguousarray(x, dtype=np.float32)
    w_qkv = np.ascontiguousarray(w_qkv, dtype=np.float32)
    w_out = np.ascontiguousarray(w_out, dtype=np.float32)
    b_out = np.ascontiguousarray(b_out, dtype=np.float32)
    x_bf = x.astype(ml_dtypes.bfloat16)
    in_maps = []
    for i in range(8):
        b, q = divmod(i, 4)
        in_maps.append(
            {
                "x_bf": x_bf[b],
                "xq_bf": np.ascontiguousarray(x_bf[b, q * NQ : (q + 1) * NQ]),
                "xq": np.ascontiguousarray(x[b, q * NQ : (q + 1) * NQ]),
                "w_qkv": w_qkv,
                "w_out": w_out,
                "b_out": b_out,
            }
        )
    return in_maps


def kernel(x, w_qkv, w_out, b_out):
    from concourse.bass_utils import run_bass_kernel_spmd

    nc = get_graph()
    in_maps = make_in_maps(x, w_qkv, w_out, b_out)
    res = run_bass_kernel_spmd(nc, in_maps, core_ids=list(range(8)))
    out = np.empty((B, N, D), np.float32)
    for i in range(8):
        b, q = divmod(i, 4)
        out[b, q * NQ : (q + 1) * NQ] = res.results[i]["out"]
    return out


# revision 28
# speedup vs baseline: 1.1105x; 1.1105x over previous
"""Trainium2 Bass kernel for log-softmax multi-head attention (8 NeuronCores).

Reference computation (per batch):
    qkv = x @ w_qkv ; q,k,v per head
    dots = scale * q @ k^T ; attn = log_softmax(dots)
    out = attn @ v  -> merge heads -> out @ w_out + b_out + x

Key algebraic identity: log_softmax is linear in the scores minus a row
constant:  attn = scale*dots - lse  with  lse_i = logsumexp_j(scale*dots_ij).
Therefore
    out_head = scale * q @ (k^T v) - lse (x) colsum(v)
which removes the O(n^2 d) attention-apply; only the lse pass is O(n^2).

Sharding: 8 cores = 2 batches x 4 query-quarters. Every core computes k/v for
its full batch (duplicated across the 4 cores of a batch) and q / lse / output
for its own 1024 query rows -> outputs are disjoint, no collectives.

Engine strategy:
- x^T arrives via big DMA xbar transposes of the bf16 copy of x (bf16 cast is
  host-side input prep; f32 x is still used for the residual).
- All n^2-sized matmuls are bf16 (1 PE cycle/row); the lse-correction rank-1
  term stays fp32; PSUM accumulation is always fp32.
- The exp+row-sum pass runs on ScalarE directly over PSUM score tiles
  (activation with accum_out), double-buffered 4+4 PSUM banks - ScalarE is the
  ~290us critical path and every other engine's work fits inside it.
"""

import numpy as np

B, N, D = 2, 4096, 512
H, DH = 8, 64
SCALE = DH**-0.5
NQ = N // 4  # own query rows per core
NT = N // 128  # 32 key tiles
QT = NQ // 128  # 8 own row tiles

_GRAPH_CACHE = {}


def _build_graph():
    import concourse.bass as bass
    import concourse.tile as tile
    from concourse import bacc, mybir
    from concourse.masks import make_identity

    f32 = mybir.dt.float32
    bf16 = mybir.dt.bfloat16
    AF = mybir.ActivationFunctionType

    nc = bacc.Bacc("TRN2", target_bir_lowering=False, debug=False)

    xbf_d = nc.dram_tensor("x_bf", [N, D], bf16, kind="ExternalInput").ap()
    xqbf_d = nc.dram_tensor("xq_bf", [NQ, D], bf16, kind="ExternalInput").ap()
    xq_d = nc.dram_tensor("xq", [NQ, D], f32, kind="ExternalInput").ap()
    wqkv_d = nc.dram_tensor("w_qkv_bf", [D, 3 * D], bf16, kind="ExternalInput").ap()
    wout_d = nc.dram_tensor("w_out_bf", [D, D], bf16, kind="ExternalInput").ap()
    bout_d = nc.dram_tensor("b_out", [D], f32, kind="ExternalInput").ap()
    out_d = nc.dram_tensor("out", [NQ, D], f32, kind="ExternalOutput").ap()

    with tile.TileContext(nc) as tc:
        with (
            tc.tile_pool(name="const", bufs=1) as const,
            tc.tile_pool(name="bigsb", bufs=1) as bigsb,
            tc.tile_pool(name="stage", bufs=3) as stage,
            tc.tile_pool(name="dout", bufs=2) as dout,
            tc.tile_pool(name="lsr", bufs=2) as lsr,
        ):
            identity = const.tile([128, 128], f32, tag="identity")
            make_identity(nc, identity[:])
            identity_bf = const.tile([128, 128], bf16, tag="identity_bf")
            make_identity(nc, identity_bf[:])
            ones_neg = const.tile([128, 1], bf16, tag="ones_neg")
            nc.vector.memset(ones_neg[:], -1.0)
            b_bc = const.tile([128, D], f32, tag="b_bc")
            nc.sync.dma_start(
                out=b_bc[:],
                in_=bass.AP(
                    tensor=bout_d.tensor,
                    offset=bout_d.offset,
                    ap=[[0, 128]] + [list(p) for p in bout_d.ap],
                ),
            )
            # weights arrive pre-cast to bf16 (host-side input prep)
            wq = []
            for j in range(4):
                w_t = const.tile([128, 3 * D], bf16, name=f"wq{j}", tag=f"wq{j}")
                nc.sync.dma_start(out=w_t[:], in_=wqkv_d[j * 128 : (j + 1) * 128, :])
                wq.append(w_t)
            wo = []
            for j in range(4):
                w_t = const.tile([128, D], bf16, name=f"wo{j}", tag=f"wo{j}")
                nc.sync.dma_start(out=w_t[:], in_=wout_d[j * 128 : (j + 1) * 128, :])
                wo.append(w_t)

            # kv_acc: [0:128, 0:512]: four [128,128] head-pair blocks of k^T v
            # (pair c's diagonal 64x64 sub-blocks are heads 2c / 2c+1).
            # [0:1, 512:1024]: -colsum(v) over all 512 v columns.
            kv_acc = const.tile([128, 1024], f32, tag="kv_acc")
            nc.vector.memset(kv_acc[:], 0.0)
            # kv_p: per-head K=128 stationary operand for OT matmuls, scaled by
            # SCALE (head h's block at rows (h%2)*64 of col block h*64, zeros
            # elsewhere so contracting against the full qT partition range only
            # picks up head h's q rows).
            kv_p = const.tile([128, 512], bf16, tag="kv_p")
            nc.vector.memset(kv_p[:], 0.0)

            kT = [bigsb.tile([128, N], bf16, name=f"kT{c}", tag=f"kT{c}") for c in range(4)]
            qT = [bigsb.tile([128, NQ], bf16, name=f"qT{c}", tag=f"qT{c}") for c in range(4)]
            xT = [bigsb.tile([128, N], bf16, name=f"xT{j}", tag=f"xT{j}") for j in range(4)]
            xTq = [bigsb.tile([128, NQ], bf16, name=f"xTq{j}", tag=f"xTq{j}") for j in range(4)]
            OT = [bigsb.tile([128, NQ], bf16, name=f"OT{c}", tag=f"OT{c}") for c in range(4)]

            lse_acc = const.tile([128, 128], f32, tag="lse_acc")
            lse_sum = const.tile([128, 64], f32, tag="lse_sum")
            lse_ln = const.tile([128, 64], bf16, tag="lse_ln")
            nvs_bf = const.tile([1, 512], bf16, tag="nvs_bf")

            # ---- phase A: x^T via big DMA xbar transposes (bf16) -------------
            # chunked by 1024 rows so phase B can start after the first chunk
            for j in range(4):
                nc.sync.dma_start(
                    out=xTq[j][:],
                    in_=xqbf_d[:, j * 128 : (j + 1) * 128],
                    transpose=True,
                )
            for r in range(4):
                for j in range(4):
                    nc.sync.dma_start(
                        out=xT[j][:, r * 1024 : (r + 1) * 1024],
                        in_=xbf_d[r * 1024 : (r + 1) * 1024, j * 128 : (j + 1) * 128],
                        transpose=True,
                    )

            # ---- phase B: qT, then k/v/kv/kT per key tile --------------------
            with tc.tile_pool(name="ab_ps", bufs=1, space="PSUM") as abps:
                for m in range(4):
                    for nn in range(2):
                        big = abps.tile([128, 1024], f32, name="big", tag="big", bufs=4)
                        for j in range(4):
                            nc.tensor.matmul(
                                big[:, 0:512],
                                lhsT=wq[j][:, m * 128 : (m + 1) * 128],
                                rhs=xTq[j][:, nn * 512 : (nn + 1) * 512],
                                start=(j == 0),
                                stop=(j == 3),
                            )
                        nc.vector.tensor_copy(
                            qT[m][:, nn * 512 : (nn + 1) * 512], big[:, 0:512]
                        )

                for t in range(NT):
                    # seg0 (0:512): k, then head-pair kv blocks, then k^T (bf16)
                    # seg1 (512:1024): v then -colsum(v) in row 0
                    big = abps.tile([128, 1024], f32, name="big", tag="big", bufs=4)
                    for half in range(2):
                        for j in range(4):
                            nc.tensor.matmul(
                                big[:, half * 512 : (half + 1) * 512],
                                lhsT=xT[j][:, t * 128 : (t + 1) * 128],
                                rhs=wq[j][:, 512 + half * 512 : 1024 + half * 512],
                                start=(j == 0),
                                stop=(j == 3),
                            )
                    ks = stage.tile([128, D], bf16, name="k_stage", tag="k_stage")
                    vs = stage.tile([128, D], bf16, name="v_stage", tag="v_stage")
                    nc.scalar.copy(ks[:], big[:, 0:512])
                    nc.scalar.copy(vs[:], big[:, 512:1024])
                    # seg0 reuse <- head-pair k^T v blocks
                    for p in range(4):
                        nc.tensor.matmul(
                            big[:, p * 128 : (p + 1) * 128],
                            lhsT=ks[:, p * 128 : (p + 1) * 128],
                            rhs=vs[:, p * 128 : (p + 1) * 128],
                            start=True,
                            stop=True,
                        )
                    # seg1 row 0 reuse <- -colsum(v)
                    nc.tensor.matmul(
                        big[0:1, 512:1024], lhsT=ones_neg[:], rhs=vs[:],
                        start=True, stop=True,
                    )
                    nc.vector.tensor_add(
                        kv_acc[:, 0:512], kv_acc[:, 0:512], big[:, 0:512]
                    )
                    nc.vector.tensor_add(
                        kv_acc[0:1, 512:1024], kv_acc[0:1, 512:1024], big[0:1, 512:1024]
                    )
                    # seg0 reuse (bf16 view, after the kv add) <- k^T
                    seg0bf = big[:, 0:512].bitcast(bf16)
                    for j in range(4):
                        nc.tensor.transpose(
                            seg0bf[:, j * 128 : (j + 1) * 128],
                            ks[:, j * 128 : (j + 1) * 128],
                            identity_bf[:],
                        )
                    for j in range(4):
                        nc.vector.tensor_copy(
                            kT[j][:, t * 128 : (t + 1) * 128],
                            seg0bf[:, j * 128 : (j + 1) * 128],
                        )

                for h in range(H):
                    r0 = (h % 2) * 64
                    nc.vector.tensor_scalar_mul(
                        kv_p[r0 : r0 + 64, h * 64 : (h + 1) * 64],
                        kv_acc[r0 : r0 + 64, (h // 2) * 128 + r0 : (h // 2) * 128 + r0 + 64],
                        SCALE,
                    )
                nc.vector.tensor_copy(nvs_bf[:], kv_acc[0:1, 512:1024])

            # residual tiles prefetched now; the DMA engines idle during C
            xr_tiles = []
            for t in range(QT):
                xr = dout.tile([128, D], f32, name=f"xr{t}", tag=f"xr{t}", bufs=1)
                nc.sync.dma_start(out=xr[:], in_=xq_d[t * 128 : (t + 1) * 128, :])
                xr_tiles.append(xr)

            # ---- phase C: scores + in-place exp + row-sums (lse) -------------
            with tc.tile_pool(name="c_ps", bufs=1, space="PSUM") as cps:
                for h in range(H):
                    r0 = (h % 2) * 64
                    c = h // 2
                    for t in range(QT):
                        for half in range(2):
                            dots = cps.tile(
                                [128, 2048], f32, name="dots", tag="dots", bufs=2
                            )
                            for cc in range(4):
                                nc.tensor.matmul(
                                    dots[:, cc * 512 : (cc + 1) * 512],
                                    lhsT=qT[c][r0 : r0 + 64, t * 128 : (t + 1) * 128],
                                    rhs=kT[c][
                                        r0 : r0 + 64,
                                        (half * 4 + cc) * 512 : (half * 4 + cc + 1) * 512,
                                    ],
                                    start=True,
                                    stop=True,
                                )
                            col = (h * 8 + t) * 2 + half
                            nc.scalar.activation(
                                out=dots[:],
                                in_=dots[:],
                                func=AF.Exp,
                                scale=SCALE,
                                accum_out=lse_acc[:, col : col + 1],
                            )
                la = lse_acc[:].rearrange("p (c two) -> p c two", two=2)
                nc.vector.tensor_add(lse_sum[:], la[:, :, 0], la[:, :, 1])
                nc.scalar.activation(out=lse_ln[:], in_=lse_sum[:], func=AF.Ln)

            # ---- phase D/E: per-head outputs, projection, residual -----------
            with tc.tile_pool(name="de_ps", bufs=1, space="PSUM") as dps:
                for h in range(H):
                    r0 = (h % 2) * 64
                    c = h // 2
                    # lse row [1, 1024] at partition 0
                    lrs = lsr.tile([1, 1024], bf16, name="lrs", tag="lrs")
                    lrp = dps.tile([128, 1024], bf16, name="lrp", tag="lrp", bufs=2)
                    for t in range(QT):
                        nc.tensor.transpose(
                            lrp[0:1, t * 128 : (t + 1) * 128],
                            lse_ln[:, h * 8 + t : h * 8 + t + 1],
                            identity_bf[:],
                        )
                    nc.vector.tensor_copy(lrs[0:1, :], lrp[0:1, :])
                    # OT_h = s (kv_h)^T q_h^T - vsum_h (x) lse_h
                    for nn in range(2):
                        ot = dps.tile([128, 512], f32, name="ot", tag="ot", bufs=4)
                        nc.tensor.matmul(
                            ot[r0 : r0 + 64, :],
                            lhsT=kv_p[:, h * 64 : (h + 1) * 64],
                            rhs=qT[c][:, nn * 512 : (nn + 1) * 512],
                            start=True,
                            stop=False,
                        )
                        nc.tensor.matmul(
                            ot[r0 : r0 + 64, :],
                            lhsT=nvs_bf[0:1, h * 64 : (h + 1) * 64],
                            rhs=lrs[0:1, nn * 512 : (nn + 1) * 512],
                            start=False,
                            stop=True,
                        )
                        nc.vector.tensor_copy(
                            OT[c][r0 : r0 + 64, nn * 512 : (nn + 1) * 512],
                            ot[r0 : r0 + 64, :],
                        )

                for t in range(QT):
                    yps = dps.tile([128, 512], f32, name="yps", tag="yps", bufs=2)
                    for c in range(4):
                        nc.tensor.matmul(
                            yps[:],
                            lhsT=OT[c][:, t * 128 : (t + 1) * 128],
                            rhs=wo[c][:],
                            start=(c == 0),
                            stop=(c == 3),
                        )
                    ysb = dout.tile([128, D], f32, name="ysb", tag="ysb")
                    nc.vector.tensor_add(ysb[:], yps[:], xr_tiles[t][:])
                    nc.vector.tensor_add(ysb[:], ysb[:], b_bc[:])
                    nc.sync.dma_start(out=out_d[t * 128 : (t + 1) * 128, :], in_=ysb[:])

    nc.compile()
    return nc


def get_graph():
    if "nc" not in _GRAPH_CACHE:
        _GRAPH_CACHE["nc"] = _build_graph()
    return _GRAPH_CACHE["nc"]


def make_in_maps(x, w_qkv, w_out, b_out):
    import ml_dtypes

    x = np.ascontiguousarray(x, dtype=np.float32)
    w_qkv = np.ascontiguousarray(w_qkv, dtype=np.float32)
    w_out = np.ascontiguousarray(w_out, dtype=np.float32)
    b_out = np.ascontiguousarray(b_out, dtype=np.float32)
    x_bf = x.astype(ml_dtypes.bfloat16)
    w_qkv_bf = w_qkv.astype(ml_dtypes.bfloat16)
    w_out_bf = w_out.astype(ml_dtypes.bfloat16)
    in_maps = []
    for i in range(8):
        b, q = divmod(i, 4)
        in_maps.append(
            {
                "x_bf": x_bf[b],
                "xq_bf": np.ascontiguousarray(x_bf[b, q * NQ : (q + 1) * NQ]),
                "xq": np.ascontiguousarray(x[b, q * NQ : (q + 1) * NQ]),
                "w_qkv_bf": w_qkv_bf,
                "w_out_bf": w_out_bf,
                "b_out": b_out,
            }
        )
    return in_maps


def kernel(x, w_qkv, w_out, b_out):
    from concourse.bass_utils import run_bass_kernel_spmd

    nc = get_graph()
    in_maps = make_in_maps(x, w_qkv, w_out, b_out)
    res = run_bass_kernel_spmd(nc, in_maps, core_ids=list(range(8)))
    out = np.empty((B, N, D), np.float32)
    for i in range(8):
        b, q = divmod(i, 4)
        out[b, q * NQ : (q + 1) * NQ] = res.results[i]["out"]
    return out
